# revision 13
# baseline (speedup 1.0000x reference)
"""CalibrationAttention Trainium2 kernel.

Data-parallel over batch across 8 NeuronCores (2 instances per core).
Self-contained: hardcodes shapes from the problem spec.

Layout strategy per instance (all fp32, matmuls in fp32r where N-chunk >= 256):
  - x [N, C] is PE-transposed to xT [C, N] (c on partitions, tiled [128, 6, 578],
    column 577 zero-padded so the 577-wide i dim splits into two 289 chunks).
  - q^T/k^T computed as w_qk^T-stationary matmuls -> qkT [128, 12, 578]
    (c3 = mo*128 + p; q rows scaled by alpha = head_scale / temperature).
  - v computed natural [n, d] -> v_sb [128, 5, 12*65] with a ones column per
    head (position 64) so P@V' also yields the softmax denominator in row 64.
  - per head: S^T = k_h^T(stationary) @ q_h^T -> psum [j, i]; heads of a pair
    live at partition offsets 0/64 so their matmuls pack into distinct PE row
    groups and run concurrently. exp on ACT (no max subtraction: logits are
    small by construction). O'^T = V'^T-stationary @ P^T accumulated over j
    tiles -> [65, i]; row 64 = denominator.
  - normalize with DVE mul by GPSIMD-partition-broadcast reciprocal; odd heads
    write via a scratch tile + SBUF->SBUF DMA partition shift (DVE lanes cannot
    shift partitions).
  - proj: attnT [c, n] is directly the stationary operand -> y [n, c_out]
    natural layout; bias added from a DRAM-broadcast tile.
"""

import os
from contextlib import ExitStack

import numpy as np

import concourse.bass as bass
import concourse.tile as tile
from concourse import bacc, mybir
from concourse._compat import with_exitstack
from concourse.masks import make_identity

F32 = mybir.dt.float32
F32R = mybir.dt.float32r

B, N, C = 16, 577, 768
H, D, HID = 12, 64, 384
P = 128
KO = C // P            # 6 c-tiles
NT = (N + P - 1) // P  # 5 n-tiles (128,128,128,128,65)
IW = 290               # i-chunk width; N padded to 580 = 2*290. fp32r ISA needs even
NPAD = 2 * IW          # free-dim counts on all matmul operands (>=256 keeps f32r fast)
ICH = (0, IW)
CCH = ((0, 512), (512, 256))  # chunking for 768-wide matmul outputs
SCALE = D ** -0.5
TMIN, TMAX = 0.5, 3.0
NCORES = 8
BPC = B // NCORES      # 2 instances per core


def _rows(mo):
    return P if mo < NT - 1 else N - (NT - 1) * P  # 65 tail


def _mrows(mo):
    return P if mo < NT - 1 else 66  # even-padded tail for fp32r matmul operands


@with_exitstack
def _emit(ctx: ExitStack, tc: tile.TileContext, io: dict, dbg: dict | None = None):
    nc = tc.nc
    AF = mybir.ActivationFunctionType
    ALU = mybir.AluOpType

    x_d = io["x"]
    qkvw_d = io["qkv_w"]
    projw_d = io["proj_w"]
    projb_d = io["proj_b"]
    tw1_d = io["t_w1"]
    tb1_d = io["t_b1"]
    tw2_d = io["t_w2"]
    tb2_d = io["t_b2"]
    y_d = io["y"]

    const = ctx.enter_context(tc.tile_pool(name="const", bufs=1))
    wqk_p = ctx.enter_context(tc.tile_pool(name="wqk", bufs=4))
    xa_p = ctx.enter_context(tc.tile_pool(name="xa", bufs=2))
    xta_p = ctx.enter_context(tc.tile_pool(name="xta", bufs=2))   # xT + attnT share slots
    qkT_p = ctx.enter_context(tc.tile_pool(name="qkT", bufs=1))
    v_p = ctx.enter_context(tc.tile_pool(name="v", bufs=1))
    pT_p = ctx.enter_context(tc.tile_pool(name="pT", bufs=2))
    y_p = ctx.enter_context(tc.tile_pool(name="y", bufs=2))
    sm_p = ctx.enter_context(tc.tile_pool(name="sm", bufs=2))
    rd_p = ctx.enter_context(tc.tile_pool(name="rd", bufs=2))
    rb_p = ctx.enter_context(tc.tile_pool(name="rb", bufs=2))
    tmp_p = ctx.enter_context(tc.tile_pool(name="tmp", bufs=2))
    ps1 = ctx.enter_context(tc.tile_pool(name="ps1", bufs=4, space="PSUM"))
    ps2 = ctx.enter_context(tc.tile_pool(name="ps2", bufs=2, space="PSUM"))

    # ---- constants ----
    wv_sb = const.tile([P, KO, C], F32R, tag="wv")
    nc.sync.dma_start(wv_sb[:], qkvw_d.bitcast(F32R)[:, 2 * C:3 * C].rearrange("(ko p) m -> p ko m", p=P))
    projw_sb = const.tile([P, KO, C], F32R, tag="pw")
    nc.sync.dma_start(projw_sb[:], projw_d.bitcast(F32R).rearrange("(ko p) m -> p ko m", p=P))
    tw1_sb = const.tile([P, KO, HID], F32, tag="tw1")
    nc.sync.dma_start(tw1_sb[:], tw1_d.rearrange("(ko p) m -> p ko m", p=P))
    tw2_sb = const.tile([P, 3, 1], F32, tag="tw2")
    nc.sync.dma_start(tw2_sb[:], tw2_d.rearrange("(ko p) m -> p ko m", p=P))
    tb1_sb = const.tile([P, 3], F32, tag="tb1")
    nc.sync.dma_start(tb1_sb[:], tb1_d.rearrange("(ko p) -> p ko", p=P))
    b2_sb = const.tile([1, 1], F32, tag="b2")
    nc.sync.dma_start(b2_sb[:], tb2_d.unsqueeze(0))
    nb2_sb = const.tile([1, 1], F32, tag="nb2")
    nc.vector.tensor_scalar_mul(nb2_sb[:], b2_sb[:], -1.0)
    pb_bc = const.tile([P, C], F32, tag="pb")
    nc.sync.dma_start(pb_bc[:], projb_d.unsqueeze(0).to_broadcast([P, C]))
    ident = const.tile([P, P], F32, tag="id")
    make_identity(nc, ident[:])

    for b in range(BPC):
        # ---- Phase T: x -> xT [128, KO, NPAD] ----
        xT = xta_p.tile([P, KO, NPAD], F32R, tag="xta")
        # zero the pad columns 577..580 (memset can't emit f32r ISA; use in*0 via DVE)
        nc.vector.tensor_scalar_mul(
            xT[:, :, N:NPAD],
            ident[:, 0:KO * (NPAD - N)].rearrange("p (a c) -> p a c", c=NPAD - N), 0.0)
        for mo in range(NT):
            rows = _rows(mo)
            xa = xa_p.tile([P, C], F32, tag="xa")
            nc.sync.dma_start(xa[:rows], x_d[b, mo * P:mo * P + rows, :])
            for ko in range(KO):
                pst = ps1.tile([P, 512], F32, tag="ps1")
                nc.tensor.transpose(pst[:P, :rows], xa[:rows, ko * P:(ko + 1) * P],
                                    ident[:rows, :rows])
                nc.vector.tensor_copy(xT[:, ko, mo * P:mo * P + rows], pst[:P, :rows])

        # ---- Phase M: temperature MLP -> alpha_bc [128, 1] ----
        hsb = sm_p.tile([P, 3], F32, tag="hsb")
        for m3 in range(3):
            hps = ps1.tile([P, 512], F32, tag="ps1")
            for ko in range(KO):
                nc.tensor.matmul(hps[:, 0:1],
                                 lhsT=tw1_sb[:, ko, m3 * P:(m3 + 1) * P],
                                 rhs=xT.bitcast(F32)[:, ko, 0:1],
                                 start=(ko == 0), stop=(ko == KO - 1))
            nc.vector.tensor_scalar(hsb[:, m3:m3 + 1], hps[:, 0:1],
                                    tb1_sb[:, m3:m3 + 1], 0.0,
                                    op0=ALU.add, op1=ALU.max)
        sps = ps1.tile([P, 512], F32, tag="ps1")
        for k3 in range(3):
            nc.tensor.matmul(sps[0:1, 0:1], lhsT=hsb[:, k3:k3 + 1],
                             rhs=tw2_sb[:, k3],
                             start=(k3 == 0), stop=(k3 == 2))
        esb = sm_p.tile([1, 1], F32, tag="esb")
        # e = exp(-(s + b2))
        nc.scalar.activation(esb[:], sps[0:1, 0:1], AF.Exp, bias=nb2_sb[:], scale=-1.0)
        dsb = sm_p.tile([1, 1], F32, tag="dsb")
        nc.vector.tensor_scalar_add(dsb[:], esb[:], 1.0)   # 1 + e  (sigmoid = 1/(1+e))
        t2 = sm_p.tile([1, 1], F32, tag="t2")
        nc.vector.reciprocal(t2[:], dsb[:])
        usb = sm_p.tile([1, 1], F32, tag="usb")
        nc.vector.tensor_scalar(usb[:], t2[:], TMAX - TMIN, TMIN, op0=ALU.mult, op1=ALU.add)
        rsb = sm_p.tile([1, 1], F32, tag="rsb")
        nc.vector.reciprocal(rsb[:], usb[:])
        asb = sm_p.tile([1, 1], F32, tag="asb")
        nc.vector.tensor_scalar_mul(asb[:], rsb[:], SCALE)  # alpha = scale / temp
        alpha_bc = sm_p.tile([P, 1], F32, tag="abc")
        nc.gpsimd.partition_broadcast(alpha_bc[:], asb[:])
        if dbg is not None and b == 0:
            nc.sync.dma_start(dbg["alpha"], alpha_bc[:])
            nc.sync.dma_start(dbg["xT"], xT.bitcast(F32)[:])

        # ---- Phase QK: qkT [128, 12, NPAD]; q rows (mo<6) scaled by alpha ----
        qkT = qkT_p.tile([P, 2 * KO, NPAD], F32R, tag="qkT")
        for mo in range(2 * KO):
            wt = wqk_p.tile([P, KO, P], F32R, tag="wqk")
            nc.sync.dma_start(wt[:], qkvw_d.bitcast(F32R)[:, mo * P:(mo + 1) * P]
                              .rearrange("(ko p) m -> p ko m", p=P))
            for i0 in ICH:
                pq = ps1.tile([P, 512], F32, tag="ps1")
                for ko in range(KO):
                    nc.tensor.matmul(pq[:, :IW],
                                     lhsT=wt[:, ko],
                                     rhs=xT[:, ko, i0:i0 + IW],
                                     start=(ko == 0), stop=(ko == KO - 1))
                if mo < KO:
                    nc.vector.tensor_scalar_mul(qkT[:, mo, i0:i0 + IW], pq[:, :IW],
                                                alpha_bc[:])
                else:
                    nc.vector.tensor_copy(qkT[:, mo, i0:i0 + IW], pq[:, :IW])

        if dbg is not None and b == 0:
            nc.sync.dma_start(dbg["qkT"], qkT.bitcast(F32)[:])

        # ---- Phase V: v_sb [128, NT, H*65] with ones col at 64 of each head ----
        vsb = v_p.tile([P, NT, H * 66], F32R, tag="v")
        for mo in range(NT):
            rows = _rows(mo)
            vv = vsb[:, mo].rearrange("p (h e) -> p h e", e=66)
            for (c0, cw) in CCH:
                pv = ps1.tile([P, 512], F32, tag="ps1")
                for ko in range(KO):
                    nc.tensor.matmul(pv[:_mrows(mo), :cw],
                                     lhsT=xT[:, ko, mo * P:mo * P + _mrows(mo)],
                                     rhs=wv_sb[:, ko, c0:c0 + cw],
                                     start=(ko == 0), stop=(ko == KO - 1))
                nc.vector.tensor_copy(
                    vv[:_mrows(mo), c0 // 64:(c0 + cw) // 64, 0:64],
                    pv[:_mrows(mo), :cw].rearrange("p (h e) -> p h e", e=64))
            # ones column at 64 of each head (in*0 + 1), zeros at 65 (in*0)
            nc.vector.tensor_scalar(vv[:, :, 64], ident[:, 0:H], 0.0, 1.0,
                                    op0=ALU.mult, op1=ALU.add)
            nc.vector.tensor_scalar_mul(vv[:, :, 65], ident[:, 0:H], 0.0)

        if dbg is not None and b == 0:
            nc.sync.dma_start(dbg["v"], vsb.bitcast(F32)[:])

        # ---- Phase A: attention ----
        aT = xta_p.tile([P, KO, NPAD], F32R, tag="xta")
        for hp in range(H // 2):
            heads = (2 * hp, 2 * hp + 1)
            pTs = [pT_p.tile([P, NT, NPAD], F32R, tag="pT", name=f"pT_{b}_{hp}_{i}")
                   for i in range(2)]
            # S^T + exp, interleaving the head pair so matmuls pack into the
            # two PE row groups (partition offsets 0 / 64).
            for jo in range(NT):
                jh = _mrows(jo)
                sp_pair = [ps2.tile([P, 2, 512], F32, tag="ps2", name=f"sp_{b}_{hp}_{jo}_{i}")
                           for i in range(2)]
                for ci, i0 in enumerate(ICH):
                    for hi, h in enumerate(heads):
                        moK, moQ, pp = KO + h // 2, h // 2, (h % 2) * 64
                        nc.tensor.matmul(
                            sp_pair[hi][:jh, ci, :IW],
                            lhsT=qkT[pp:pp + 64, moK, jo * P:jo * P + jh],
                            rhs=qkT[pp:pp + 64, moQ, i0:i0 + IW],
                            start=True, stop=True)
                for hi in range(2):
                    if jo == NT - 1:
                        # rows 64:66 pre-zeroed (base-64 aligned); exp then
                        # rewrites row 64 (j=576) and leaves pad row 65 zero
                        nc.vector.tensor_scalar_mul(pTs[hi][64:66, jo, :],
                                                    pb_bc[64:66, 0:NPAD], 0.0)
                    je = _rows(jo)
                    nc.scalar.activation(
                        pTs[hi][:je, jo].rearrange("p (c w) -> p c w", w=IW),
                        sp_pair[hi][:je, :, :IW], AF.Exp)
            # P @ V' (accumulate over j tiles; row 64 = sum of P = denominator)
            for hi, h in enumerate(heads):
                ochunks = []
                for ci, i0 in enumerate(ICH):
                    po = ps1.tile([P, 512], F32, tag="ps1")
                    for jo in range(NT):
                        jh = _mrows(jo)
                        nc.tensor.matmul(
                            po[:66, :IW],
                            lhsT=vsb[:jh, jo, h * 66:(h + 1) * 66],
                            rhs=pTs[hi][:jh, jo, i0:i0 + IW],
                            start=(jo == 0), stop=(jo == NT - 1))
                    ochunks.append(po)
                # normalize: attnT rows (h%2)*64 .. +64 of c-tile h//2
                rden = rd_p.tile([P, NPAD], F32, tag="rd")
                for ci, i0 in enumerate(ICH):
                    nc.vector.reciprocal(rden[64:65, i0:i0 + IW], ochunks[ci][64:65, :IW])
                # partition_broadcast requires its input at partition 0 (HW
                # reads garbage from base-64 APs) -> DMA-shift row 64 -> row 0
                rden0 = rd_p.tile([1, NPAD], F32, tag="rd0")
                nc.sync.dma_start(rden0[0:1, :], rden[64:65, :])
                rb = rb_p.tile([P, NPAD], F32, tag="rb")
                nc.gpsimd.partition_broadcast(rb[:], rden0[0:1, :])
                mo6 = h // 2
                if h % 2 == 0:
                    for ci, i0 in enumerate(ICH):
                        nc.vector.tensor_mul(aT[0:64, mo6, i0:i0 + IW],
                                             ochunks[ci][0:64, :IW], rb[0:64, i0:i0 + IW])
                else:
                    tshift = tmp_p.tile([64, NPAD], F32R, tag="tmp")
                    for ci, i0 in enumerate(ICH):
                        nc.vector.tensor_mul(tshift[0:64, i0:i0 + IW],
                                             ochunks[ci][0:64, :IW], rb[0:64, i0:i0 + IW])
                    nc.sync.dma_start(aT[64:128, mo6, :], tshift[0:64, :])

        if dbg is not None and b == 0:
            nc.sync.dma_start(dbg["aT"], aT.bitcast(F32)[:])

        # ---- Phase P: proj + bias -> y ----
        for mo in range(NT):
            rows = _rows(mo)
            ysb = y_p.tile([P, C], F32, tag="y")
            for (c0, cw) in CCH:
                pp_ = ps1.tile([P, 512], F32, tag="ps1")
                for ko in range(KO):
                    nc.tensor.matmul(pp_[:_mrows(mo), :cw],
                                     lhsT=aT[:, ko, mo * P:mo * P + _mrows(mo)],
                                     rhs=projw_sb[:, ko, c0:c0 + cw],
                                     start=(ko == 0), stop=(ko == KO - 1))
                nc.vector.tensor_add(ysb[:rows, c0:c0 + cw], pp_[:rows, :cw],
                                     pb_bc[:rows, c0:c0 + cw])
            nc.sync.dma_start(y_d[b, mo * P:mo * P + rows, :], ysb[:rows])


def build(debug=False):
    """Build and compile the per-core Bass module. Returns nc."""
    nc = bacc.Bacc("TRN2", target_bir_lowering=False, debug=False,
                   enable_asserts=False, num_devices=NCORES)
    io = {}
    io["x"] = nc.dram_tensor("x", [BPC, N, C], F32, kind="ExternalInput").ap()
    io["qkv_w"] = nc.dram_tensor("qkv_w", [C, 3 * C], F32, kind="ExternalInput").ap()
    io["proj_w"] = nc.dram_tensor("proj_w", [C, C], F32, kind="ExternalInput").ap()
    io["proj_b"] = nc.dram_tensor("proj_b", [C], F32, kind="ExternalInput").ap()
    io["t_w1"] = nc.dram_tensor("t_w1", [C, HID], F32, kind="ExternalInput").ap()
    io["t_b1"] = nc.dram_tensor("t_b1", [HID], F32, kind="ExternalInput").ap()
    io["t_w2"] = nc.dram_tensor("t_w2", [HID, 1], F32, kind="ExternalInput").ap()
    io["t_b2"] = nc.dram_tensor("t_b2", [1], F32, kind="ExternalInput").ap()
    io["y"] = nc.dram_tensor("y", [BPC, N, C], F32, kind="ExternalOutput").ap()

    dbg = None
    if debug:
        dbg = {
            "alpha": nc.dram_tensor("dbg_alpha", [P, 1], F32, kind="ExternalOutput").ap(),
            "xT": nc.dram_tensor("dbg_xT", [P, KO, NPAD], F32, kind="ExternalOutput").ap(),
            "qkT": nc.dram_tensor("dbg_qkT", [P, 2 * KO, NPAD], F32, kind="ExternalOutput").ap(),
            "v": nc.dram_tensor("dbg_v", [P, NT, H * 66], F32, kind="ExternalOutput").ap(),
            "aT": nc.dram_tensor("dbg_aT", [P, KO, NPAD], F32, kind="ExternalOutput").ap(),
        }
    with tile.TileContext(nc) as tc:
        _emit(tc, io, dbg)
    nc.compile()
    return nc


_NC_CACHE = None


def _get_nc():
    global _NC_CACHE
    if _NC_CACHE is None:
        _NC_CACHE = build()
    return _NC_CACHE


def make_in_maps(inputs: dict) -> list[dict]:
    ws = {k: np.ascontiguousarray(np.asarray(v, dtype=np.float32))
          for k, v in inputs.items() if k != "x"}
    x = np.asarray(inputs["x"], dtype=np.float32)
    return [dict(ws, x=np.ascontiguousarray(x[i * BPC:(i + 1) * BPC]))
            for i in range(NCORES)]


def kernel(**inputs) -> np.ndarray:
    from concourse.bass_utils import run_bass_kernel_spmd
    nc = _get_nc()
    in_maps = make_in_maps(inputs)
    res = run_bass_kernel_spmd(nc, in_maps, core_ids=list(range(NCORES)))
    return np.concatenate([r["y"] for r in res.results], axis=0)


if __name__ == "__main__":
    rng = np.random.default_rng(0)
    ins = {
        "x": rng.standard_normal((B, N, C), dtype=np.float32),
        "qkv_w": (rng.standard_normal((C, 3 * C)) * 0.02).astype(np.float32),
        "proj_w": (rng.standard_normal((C, C)) * 0.02).astype(np.float32),
        "proj_b": np.zeros(C, np.float32),
        "t_w1": (rng.standard_normal((C, HID)) * 0.02).astype(np.float32),
        "t_b1": np.zeros(HID, np.float32),
        "t_w2": (rng.standard_normal((HID, 1)) * 0.02).astype(np.float32),
        "t_b2": np.zeros(1, np.float32),
    }
    out = kernel(**ins)
    print("out", out.shape, out.dtype, float(np.abs(out).max()))


# revision 30
# speedup vs baseline: 1.0368x; 1.0368x over previous
"""CalibrationAttention Trainium2 kernel.

Data-parallel over batch across 8 NeuronCores (2 instances per core).
Self-contained: hardcodes shapes from the problem spec.

Layout strategy per instance (fp32 everywhere; matmuls run in fp32r, which is
full PE rate for moving dims >= 256 and needs even operand free-dim counts):
  - x [N, C] is PE-transposed to xT [C, N] (c on partitions, tiled [128, 6, 580],
    columns 577..580 zero-padded so the i dim splits into two 290-wide chunks).
  - q^T/k^T computed as w_qk^T-stationary matmuls -> qkT [128, 12, 580]
    (c3 = mo*128 + p; q rows scaled by alpha = head_scale / temperature).
  - v computed natural [n, d] -> v_sb [128, 5, 12*66]: per head, col 64 = ones
    (so P@V' also yields the softmax denominator in row 64), col 65 = zeros
    (even-M padding for fp32r).
  - per head: S^T = k_h^T(stationary) @ q_h^T -> psum [j, i]; heads of a pair
    live at partition offsets 0/64 so their matmuls pack into distinct PE row
    groups and run concurrently. exp on ACT (no max subtraction: logits are
    small by construction). O'^T = V'^T-stationary @ P^T accumulated over j
    tiles -> [65, i]; row 64 = denominator.
  - normalize with DVE mul by GPSIMD-partition-broadcast reciprocal; odd heads
    write via a scratch tile + SBUF->SBUF DMA partition shift (DVE lanes cannot
    shift partitions).
  - proj: attnT [c, n] is directly the stationary operand -> y [n, c_out]
    natural layout; bias added from a DRAM-broadcast tile.
"""

import os
from contextlib import ExitStack

import numpy as np

import concourse.bass as bass
import concourse.tile as tile
from concourse import bacc, mybir
from concourse._compat import with_exitstack
from concourse.masks import make_identity

F32 = mybir.dt.float32
F32R = mybir.dt.float32r

B, N, C = 16, 577, 768
H, D, HID = 12, 64, 384
P = 128
KO = C // P            # 6 c-tiles
NT = (N + P - 1) // P  # 5 n-tiles (128,128,128,128,65)
IW = 290               # i-chunk width; N padded to 580 = 2*290. fp32r ISA needs even
NPAD = 2 * IW          # free-dim counts on all matmul operands (>=256 keeps f32r fast)
ICH = (0, IW)
CCH = ((0, 512), (512, 256))  # chunking for 768-wide matmul outputs
SCALE = D ** -0.5
TMIN, TMAX = 0.5, 3.0
NCORES = 8
BPC = B // NCORES      # 2 instances per core


def _rows(mo):
    return P if mo < NT - 1 else N - (NT - 1) * P  # 65 tail


def _mrows(mo):
    return P if mo < NT - 1 else 66  # even-padded tail for fp32r matmul operands


@with_exitstack
def _emit(ctx: ExitStack, tc: tile.TileContext, io: dict, dbg: dict | None = None):
    nc = tc.nc
    AF = mybir.ActivationFunctionType
    ALU = mybir.AluOpType

    x_d = io["x"]
    qkvw_d = io["qkv_w"]
    projw_d = io["proj_w"]
    projb_d = io["proj_b"]
    tw1_d = io["t_w1"]
    tb1_d = io["t_b1"]
    tw2_d = io["t_w2"]
    tb2_d = io["t_b2"]
    y_d = io["y"]

    const = ctx.enter_context(tc.tile_pool(name="const", bufs=1))
    wqk_p = ctx.enter_context(tc.tile_pool(name="wqk", bufs=4))
    xa_p = ctx.enter_context(tc.tile_pool(name="xa", bufs=2))
    xT_pool = ctx.enter_context(tc.tile_pool(name="xTp", bufs=1))
    aT_pool = ctx.enter_context(tc.tile_pool(name="aTp", bufs=2))
    qkT_p = ctx.enter_context(tc.tile_pool(name="qkT", bufs=1))
    v_p = ctx.enter_context(tc.tile_pool(name="v", bufs=2))
    pT_p = ctx.enter_context(tc.tile_pool(name="pT", bufs=2))
    y_p = ctx.enter_context(tc.tile_pool(name="y", bufs=2))
    sm_p = ctx.enter_context(tc.tile_pool(name="sm", bufs=2))
    rd_p = ctx.enter_context(tc.tile_pool(name="rd", bufs=1))
    rb_p = ctx.enter_context(tc.tile_pool(name="rb", bufs=1))
    tmp_p = ctx.enter_context(tc.tile_pool(name="tmp", bufs=1))
    ps1 = ctx.enter_context(tc.tile_pool(name="ps1", bufs=4, space="PSUM"))
    ps2 = ctx.enter_context(tc.tile_pool(name="ps2", bufs=2, space="PSUM"))

    # ---- constants ----
    wv_sb = const.tile([P, KO, C], F32R, tag="wv")
    nc.sync.dma_start(wv_sb[:], qkvw_d.bitcast(F32R)[:, 2 * C:3 * C].rearrange("(ko p) m -> p ko m", p=P))
    projw_sb = const.tile([P, KO, C], F32R, tag="pw")
    nc.sync.dma_start(projw_sb[:], projw_d.bitcast(F32R).rearrange("(ko p) m -> p ko m", p=P))
    tw1_sb = const.tile([P, KO, HID], F32, tag="tw1")
    nc.sync.dma_start(tw1_sb[:], tw1_d.rearrange("(ko p) m -> p ko m", p=P))
    tw2_sb = const.tile([P, 3, 1], F32, tag="tw2")
    nc.sync.dma_start(tw2_sb[:], tw2_d.rearrange("(ko p) m -> p ko m", p=P))
    tb1_sb = const.tile([P, 3], F32, tag="tb1")
    nc.sync.dma_start(tb1_sb[:], tb1_d.rearrange("(ko p) -> p ko", p=P))
    b2_sb = const.tile([1, 1], F32, tag="b2")
    nc.sync.dma_start(b2_sb[:], tb2_d.unsqueeze(0))
    nb2_sb = const.tile([1, 1], F32, tag="nb2")
    nc.vector.tensor_scalar_mul(nb2_sb[:], b2_sb[:], -1.0)
    pb_bc = const.tile([P, C], F32, tag="pb")
    nc.sync.dma_start(pb_bc[:], projb_d.unsqueeze(0).to_broadcast([P, C]))
    ident = const.tile([P, P], F32, tag="id")
    make_identity(nc, ident[:])

    def phase_T(b):
        """x -> xT [128, KO, NPAD] via PE transpose."""
        xT = xT_pool.tile([P, KO, NPAD], F32R, tag="xT", name=f"xT_{b}")
        # zero the pad columns 577..580 (memset can't emit f32r ISA; use in*0 via DVE)
        nc.vector.tensor_scalar_mul(
            xT[:, :, N:NPAD],
            ident[:, 0:KO * (NPAD - N)].rearrange("p (a c) -> p a c", c=NPAD - N), 0.0)
        for mo in range(NT):
            rows = _rows(mo)
            xa = xa_p.tile([P, C], F32, tag="xa", name=f"xa_{b}_{mo}")
            nc.sync.dma_start(xa[:rows], x_d[b, mo * P:mo * P + rows, :])
            for ko in range(KO):
                pst = ps1.tile([P, 512], F32, tag="ps1", name=f"pst_{b}_{mo}_{ko}")
                nc.tensor.transpose(pst[:P, :rows], xa[:rows, ko * P:(ko + 1) * P],
                                    ident[:rows, :rows])
                nc.vector.tensor_copy(xT[:, ko, mo * P:mo * P + rows], pst[:P, :rows])
        return xT

    def phase_M(b, xT):
        """temperature MLP -> alpha_bc [128, 1]."""
        hsb = sm_p.tile([P, 3], F32, tag="hsb", name=f"hsb_{b}")
        for m3 in range(3):
            hps = ps1.tile([P, 512], F32, tag="ps1", name=f"hps_{b}_{m3}")
            for ko in range(KO):
                nc.tensor.matmul(hps[:, 0:1],
                                 lhsT=tw1_sb[:, ko, m3 * P:(m3 + 1) * P],
                                 rhs=xT.bitcast(F32)[:, ko, 0:1],
                                 start=(ko == 0), stop=(ko == KO - 1))
            nc.vector.tensor_scalar(hsb[:, m3:m3 + 1], hps[:, 0:1],
                                    tb1_sb[:, m3:m3 + 1], 0.0,
                                    op0=ALU.add, op1=ALU.max)
        sps = ps1.tile([P, 512], F32, tag="ps1", name=f"sps_{b}")
        for k3 in range(3):
            nc.tensor.matmul(sps[0:1, 0:1], lhsT=hsb[:, k3:k3 + 1],
                             rhs=tw2_sb[:, k3],
                             start=(k3 == 0), stop=(k3 == 2))
        esb = sm_p.tile([1, 1], F32, tag="esb", name=f"esb_{b}")
        # e = exp(-(s + b2)); sigmoid = 1/(1+e)
        nc.scalar.activation(esb[:], sps[0:1, 0:1], AF.Exp, bias=nb2_sb[:], scale=-1.0)
        dsb = sm_p.tile([1, 1], F32, tag="dsb", name=f"dsb_{b}")
        nc.vector.tensor_scalar_add(dsb[:], esb[:], 1.0)
        t2 = sm_p.tile([1, 1], F32, tag="t2", name=f"t2_{b}")
        nc.vector.reciprocal(t2[:], dsb[:])
        usb = sm_p.tile([1, 1], F32, tag="usb", name=f"usb_{b}")
        nc.vector.tensor_scalar(usb[:], t2[:], TMAX - TMIN, TMIN, op0=ALU.mult, op1=ALU.add)
        rsb = sm_p.tile([1, 1], F32, tag="rsb", name=f"rsb_{b}")
        nc.vector.reciprocal(rsb[:], usb[:])
        asb = sm_p.tile([1, 1], F32, tag="asb", name=f"asb_{b}")
        nc.vector.tensor_scalar_mul(asb[:], rsb[:], SCALE)  # alpha = scale / temp
        alpha_bc = sm_p.tile([P, 1], F32, tag="abc", name=f"abc_{b}")
        nc.gpsimd.partition_broadcast(alpha_bc[:], asb[:])
        if dbg is not None and b == 0:
            nc.sync.dma_start(dbg["alpha"], alpha_bc[:])
            nc.sync.dma_start(dbg["xT"], xT.bitcast(F32)[:])
        return alpha_bc

    def phase_QK(b, xT, alpha_bc):
        """qkT [128, 12, NPAD]; q rows (mo<6) scaled by alpha."""
        qkT = qkT_p.tile([P, 2 * KO, NPAD], F32R, tag="qkT", name=f"qkT_{b}")
        for mo in range(2 * KO):
            wt = wqk_p.tile([P, KO, P], F32R, tag="wqk", name=f"wt_{b}_{mo}")
            nc.sync.dma_start(wt[:], qkvw_d.bitcast(F32R)[:, mo * P:(mo + 1) * P]
                              .rearrange("(ko p) m -> p ko m", p=P))
            for i0 in ICH:
                pq = ps1.tile([P, 512], F32, tag="ps1", name=f"pq_{b}_{mo}_{i0}")
                for ko in range(KO):
                    nc.tensor.matmul(pq[:, :IW],
                                     lhsT=wt[:, ko],
                                     rhs=xT[:, ko, i0:i0 + IW],
                                     start=(ko == 0), stop=(ko == KO - 1))
                if mo < KO:
                    nc.vector.tensor_scalar_mul(qkT[:, mo, i0:i0 + IW], pq[:, :IW],
                                                alpha_bc[:])
                else:
                    nc.vector.tensor_copy(qkT[:, mo, i0:i0 + IW], pq[:, :IW])
        if dbg is not None and b == 0:
            nc.sync.dma_start(dbg["qkT"], qkT.bitcast(F32)[:])
        return qkT

    def gen_V(b, xT, out):
        """v_sb [128, NT, H*66]: ones col at 64, zeros at 65 of each head."""
        vsb = v_p.tile([P, NT, H * 66], F32R, tag="v", name=f"v_{b}")
        out["v"] = vsb
        for mo in range(NT):
            vv = vsb[:, mo].rearrange("p (h e) -> p h e", e=66)
            for (c0, cw) in CCH:
                pv = ps1.tile([P, 512], F32, tag="ps1", name=f"pv_{b}_{mo}_{c0}")
                for ko in range(KO):
                    nc.tensor.matmul(pv[:_mrows(mo), :cw],
                                     lhsT=xT[:, ko, mo * P:mo * P + _mrows(mo)],
                                     rhs=wv_sb[:, ko, c0:c0 + cw],
                                     start=(ko == 0), stop=(ko == KO - 1))
                nc.vector.tensor_copy(
                    vv[:_mrows(mo), c0 // 64:(c0 + cw) // 64, 0:64],
                    pv[:_mrows(mo), :cw].rearrange("p (h e) -> p h e", e=64))
            nc.vector.tensor_scalar(vv[:, :, 64], ident[:, 0:H], 0.0, 1.0,
                                    op0=ALU.mult, op1=ALU.add)
            nc.vector.tensor_scalar_mul(vv[:, :, 65], ident[:, 0:H], 0.0)
            yield
        if dbg is not None and b == 0:
            nc.sync.dma_start(dbg["v"], vsb.bitcast(F32)[:])

    def gen_A(b, qkT, vsb, out):
        """attention -> aT [128, KO, NPAD]; yields after each head pair."""
        aT = aT_pool.tile([P, KO, NPAD], F32R, tag="aT", name=f"aT_{b}")
        out["aT"] = aT
        for hp in range(H // 2):
            heads = (2 * hp, 2 * hp + 1)
            pTs = [pT_p.tile([P, NT, NPAD], F32R, tag="pT", name=f"pT_{b}_{hp}_{i}")
                   for i in range(2)]
            # S^T + exp; the head pair occupies PE row groups 0 / 64 and the
            # matmuls are interleaved so they run concurrently in the array.
            for jo in range(NT):
                jh = _mrows(jo)
                sp_pair = [ps2.tile([P, 2, 512], F32, tag="ps2", name=f"sp_{b}_{hp}_{jo}_{i}")
                           for i in range(2)]
                for ci, i0 in enumerate(ICH):
                    for hi, h in enumerate(heads):
                        moK, moQ, pp = KO + h // 2, h // 2, (h % 2) * 64
                        nc.tensor.matmul(
                            sp_pair[hi][:jh, ci, :IW],
                            lhsT=qkT[pp:pp + 64, moK, jo * P:jo * P + jh],
                            rhs=qkT[pp:pp + 64, moQ, i0:i0 + IW],
                            start=True, stop=True)
                for hi in range(2):
                    if jo == NT - 1:
                        # rows 64:66 pre-zeroed (base-64 aligned); exp then
                        # rewrites row 64 (j=576) and leaves pad row 65 zero
                        nc.vector.tensor_scalar_mul(pTs[hi][64:66, jo, :],
                                                    pb_bc[64:66, 0:NPAD], 0.0)
                    je = _rows(jo)
                    nc.scalar.activation(
                        pTs[hi][:je, jo].rearrange("p (c w) -> p c w", w=IW),
                        sp_pair[hi][:je, :, :IW], AF.Exp)
            # P @ V' (accumulate over j tiles; row 64 = sum of P = denominator)
            for hi, h in enumerate(heads):
                ochunks = []
                for ci, i0 in enumerate(ICH):
                    po = ps1.tile([P, 512], F32, tag="ps1", name=f"po_{b}_{h}_{ci}")
                    for jo in range(NT):
                        jh = _mrows(jo)
                        nc.tensor.matmul(
                            po[:66, :IW],
                            lhsT=vsb[:jh, jo, h * 66:(h + 1) * 66],
                            rhs=pTs[hi][:jh, jo, i0:i0 + IW],
                            start=(jo == 0), stop=(jo == NT - 1))
                    ochunks.append(po)
                # normalize: attnT rows (h%2)*64 .. +64 of c-tile h//2
                rden = rd_p.tile([P, NPAD], F32, tag="rd", name=f"rden_{b}_{h}")
                for ci, i0 in enumerate(ICH):
                    nc.vector.reciprocal(rden[64:65, i0:i0 + IW], ochunks[ci][64:65, :IW])
                # partition_broadcast requires its input at partition 0 (HW
                # reads garbage from base-64 APs) -> DMA-shift row 64 -> row 0
                rden0 = rd_p.tile([1, NPAD], F32, tag="rd0", name=f"rden0_{b}_{h}")
                nc.sync.dma_start(rden0[0:1, :], rden[64:65, :])
                rb = rb_p.tile([P, NPAD], F32, tag="rb", name=f"rb_{b}_{h}")
                nc.gpsimd.partition_broadcast(rb[:], rden0[0:1, :])
                mo6 = h // 2
                if h % 2 == 0:
                    for ci, i0 in enumerate(ICH):
                        nc.vector.tensor_mul(aT[0:64, mo6, i0:i0 + IW],
                                             ochunks[ci][0:64, :IW], rb[0:64, i0:i0 + IW])
                else:
                    tshift = tmp_p.tile([64, NPAD], F32R, tag="tmp", name=f"tsh_{b}_{h}")
                    for ci, i0 in enumerate(ICH):
                        nc.vector.tensor_mul(tshift[0:64, i0:i0 + IW],
                                             ochunks[ci][0:64, :IW], rb[0:64, i0:i0 + IW])
                    nc.sync.dma_start(aT[64:128, mo6, :], tshift[0:64, :])
            yield
        if dbg is not None and b == 0:
            nc.sync.dma_start(dbg["aT"], aT.bitcast(F32)[:])

    def gen_P(b, aT):
        """proj + bias -> y; yields after each n-tile."""
        for mo in range(NT):
            rows = _rows(mo)
            ysb = y_p.tile([P, C], F32, tag="y", name=f"y_{b}_{mo}")
            for (c0, cw) in CCH:
                pp_ = ps1.tile([P, 512], F32, tag="ps1", name=f"yp_{b}_{mo}_{c0}")
                for ko in range(KO):
                    nc.tensor.matmul(pp_[:_mrows(mo), :cw],
                                     lhsT=aT[:, ko, mo * P:mo * P + _mrows(mo)],
                                     rhs=projw_sb[:, ko, c0:c0 + cw],
                                     start=(ko == 0), stop=(ko == KO - 1))
                nc.vector.tensor_add(ysb[:rows, c0:c0 + cw], pp_[:rows, :cw],
                                     pb_bc[:rows, c0:c0 + cw])
            nc.sync.dma_start(y_d[b, mo * P:mo * P + rows, :], ysb[:rows])
            yield

    def drain(g):
        for _ in g:
            pass

    # Interleaved schedule: instance 1's V fills the PE while instance 0's
    # attention is ACT(exp)-bound, and instance 0's proj fills instance 1's
    # attention the same way.
    st0, st1 = {}, {}
    xT0 = phase_T(0)
    a0 = phase_M(0, xT0)
    qkT0 = phase_QK(0, xT0, a0)
    drain(gen_V(0, xT0, st0))
    xT1 = phase_T(1)
    a1 = phase_M(1, xT1)

    gA0 = gen_A(0, qkT0, st0["v"], st0)
    gV1 = gen_V(1, xT1, st1)
    while next(gA0, "end") != "end":
        next(gV1, None)
    drain(gV1)

    qkT1 = phase_QK(1, xT1, a1)

    gA1 = gen_A(1, qkT1, st1["v"], st1)
    gP0 = gen_P(0, st0["aT"])
    while next(gA1, "end") != "end":
        next(gP0, None)
    drain(gP0)
    drain(gen_P(1, st1["aT"]))


def build(debug=False):
    """Build and compile the per-core Bass module. Returns nc."""
    nc = bacc.Bacc("TRN2", target_bir_lowering=False, debug=False,
                   enable_asserts=False, num_devices=NCORES)
    io = {}
    io["x"] = nc.dram_tensor("x", [BPC, N, C], F32, kind="ExternalInput").ap()
    io["qkv_w"] = nc.dram_tensor("qkv_w", [C, 3 * C], F32, kind="ExternalInput").ap()
    io["proj_w"] = nc.dram_tensor("proj_w", [C, C], F32, kind="ExternalInput").ap()
    io["proj_b"] = nc.dram_tensor("proj_b", [C], F32, kind="ExternalInput").ap()
    io["t_w1"] = nc.dram_tensor("t_w1", [C, HID], F32, kind="ExternalInput").ap()
    io["t_b1"] = nc.dram_tensor("t_b1", [HID], F32, kind="ExternalInput").ap()
    io["t_w2"] = nc.dram_tensor("t_w2", [HID, 1], F32, kind="ExternalInput").ap()
    io["t_b2"] = nc.dram_tensor("t_b2", [1], F32, kind="ExternalInput").ap()
    io["y"] = nc.dram_tensor("y", [BPC, N, C], F32, kind="ExternalOutput").ap()

    dbg = None
    if debug:
        dbg = {
            "alpha": nc.dram_tensor("dbg_alpha", [P, 1], F32, kind="ExternalOutput").ap(),
            "xT": nc.dram_tensor("dbg_xT", [P, KO, NPAD], F32, kind="ExternalOutput").ap(),
            "qkT": nc.dram_tensor("dbg_qkT", [P, 2 * KO, NPAD], F32, kind="ExternalOutput").ap(),
            "v": nc.dram_tensor("dbg_v", [P, NT, H * 66], F32, kind="ExternalOutput").ap(),
            "aT": nc.dram_tensor("dbg_aT", [P, KO, NPAD], F32, kind="ExternalOutput").ap(),
        }
    with tile.TileContext(nc) as tc:
        _emit(tc, io, dbg)
    nc.compile()
    return nc


_NC_CACHE = None


def _get_nc():
    global _NC_CACHE
    if _NC_CACHE is None:
        _NC_CACHE = build()
    return _NC_CACHE


def make_in_maps(inputs: dict) -> list[dict]:
    ws = {k: np.ascontiguousarray(np.asarray(v, dtype=np.float32))
          for k, v in inputs.items() if k != "x"}
    x = np.asarray(inputs["x"], dtype=np.float32)
    return [dict(ws, x=np.ascontiguousarray(x[i * BPC:(i + 1) * BPC]))
            for i in range(NCORES)]


def kernel(**inputs) -> np.ndarray:
    from concourse.bass_utils import run_bass_kernel_spmd
    nc = _get_nc()
    in_maps = make_in_maps(inputs)
    res = run_bass_kernel_spmd(nc, in_maps, core_ids=list(range(NCORES)))
    return np.concatenate([r["y"] for r in res.results], axis=0)


if __name__ == "__main__":
    rng = np.random.default_rng(0)
    ins = {
        "x": rng.standard_normal((B, N, C), dtype=np.float32),
        "qkv_w": (rng.standard_normal((C, 3 * C)) * 0.02).astype(np.float32),
        "proj_w": (rng.standard_normal((C, C)) * 0.02).astype(np.float32),
        "proj_b": np.zeros(C, np.float32),
        "t_w1": (rng.standard_normal((C, HID)) * 0.02).astype(np.float32),
        "t_b1": np.zeros(HID, np.float32),
        "t_w2": (rng.standard_normal((HID, 1)) * 0.02).astype(np.float32),
        "t_b2": np.zeros(1, np.float32),
    }
    out = kernel(**ins)
    print("out", out.shape, out.dtype, float(np.abs(out).max()))


# revision 43
# speedup vs baseline: 1.0587x; 1.0211x over previous
"""CalibrationAttention Trainium2 kernel.

Data-parallel over batch across 8 NeuronCores (2 instances per core).
Self-contained: hardcodes shapes from the problem spec.

Layout strategy per instance (fp32 everywhere; matmuls run in fp32r, which is
full PE rate for moving dims >= 256 and needs even operand free-dim counts):
  - x [N, C] is PE-transposed to xT [C, N] (c on partitions, tiled [128, 6, 580],
    columns 577..580 zero-padded so the i dim splits into two 290-wide chunks).
  - q^T/k^T computed as w_qk^T-stationary matmuls -> qkT [128, 12, 580]
    (c3 = mo*128 + p; q rows scaled by alpha = head_scale / temperature).
  - v computed natural [n, d] -> v_sb [128, 5, 12*66]: per head, col 64 = ones
    (so P@V' also yields the softmax denominator in row 64), col 65 = zeros
    (even-M padding for fp32r).
  - per head: S^T = k_h^T(stationary) @ q_h^T -> psum [j, i]; heads of a pair
    live at partition offsets 0/64 so their matmuls pack into distinct PE row
    groups and run concurrently. exp on ACT (no max subtraction: logits are
    small by construction). O'^T = V'^T-stationary @ P^T accumulated over j
    tiles -> [65, i]; row 64 = denominator.
  - normalize with DVE mul by GPSIMD-partition-broadcast reciprocal; odd heads
    write via a scratch tile + SBUF->SBUF DMA partition shift (DVE lanes cannot
    shift partitions).
  - proj: attnT [c, n] is directly the stationary operand -> y [n, c_out]
    natural layout; bias added from a DRAM-broadcast tile.
"""

import os
from contextlib import ExitStack

import numpy as np

import concourse.bass as bass
import concourse.tile as tile
from concourse import bacc, mybir
from concourse._compat import with_exitstack
from concourse.masks import make_identity

F32 = mybir.dt.float32
F32R = mybir.dt.float32r

B, N, C = 16, 577, 768
H, D, HID = 12, 64, 384
P = 128
KO = C // P            # 6 c-tiles
NT = (N + P - 1) // P  # 5 n-tiles (128,128,128,128,65)
IW = 290               # i-chunk width; N padded to 580 = 2*290. fp32r ISA needs even
NPAD = 2 * IW          # free-dim counts on all matmul operands (>=256 keeps f32r fast)
ICH = (0, IW)
CCH = ((0, 512), (512, 256))  # chunking for 768-wide matmul outputs
SCALE = D ** -0.5
TMIN, TMAX = 0.5, 3.0
NCORES = 8
BPC = B // NCORES      # 2 instances per core


def _rows(mo):
    return P if mo < NT - 1 else N - (NT - 1) * P  # 65 tail


def _mrows(mo):
    return P if mo < NT - 1 else 66  # even-padded tail for fp32r matmul operands


@with_exitstack
def _emit(ctx: ExitStack, tc: tile.TileContext, io: dict, dbg: dict | None = None):
    nc = tc.nc
    AF = mybir.ActivationFunctionType
    ALU = mybir.AluOpType

    x_d = io["x"]
    qkvw_d = io["qkv_w"]
    projw_d = io["proj_w"]
    projb_d = io["proj_b"]
    tw1_d = io["t_w1"]
    tb1_d = io["t_b1"]
    tw2_d = io["t_w2"]
    tb2_d = io["t_b2"]
    y_d = io["y"]

    const = ctx.enter_context(tc.tile_pool(name="const", bufs=1))
    wqk_p = ctx.enter_context(tc.tile_pool(name="wqk", bufs=4))
    xa_p = ctx.enter_context(tc.tile_pool(name="xa", bufs=3))
    xT_pool = ctx.enter_context(tc.tile_pool(name="xTp", bufs=1))
    aT_pool = ctx.enter_context(tc.tile_pool(name="aTp", bufs=2))
    qkT_p = ctx.enter_context(tc.tile_pool(name="qkT", bufs=1))
    v_p = ctx.enter_context(tc.tile_pool(name="v", bufs=2))
    pT_p = ctx.enter_context(tc.tile_pool(name="pT", bufs=2))
    y_p = ctx.enter_context(tc.tile_pool(name="y", bufs=2))
    sm_p = ctx.enter_context(tc.tile_pool(name="sm", bufs=2))
    rd_p = ctx.enter_context(tc.tile_pool(name="rd", bufs=1))
    rb_p = ctx.enter_context(tc.tile_pool(name="rb", bufs=1))
    tmp_p = ctx.enter_context(tc.tile_pool(name="tmp", bufs=1))
    ps1 = ctx.enter_context(tc.tile_pool(name="ps1", bufs=4, space="PSUM"))
    ps2 = ctx.enter_context(tc.tile_pool(name="ps2", bufs=2, space="PSUM"))

    # ---- constants ----
    wv_sb = const.tile([P, KO, C], F32R, tag="wv")
    nc.sync.dma_start(wv_sb[:], qkvw_d.bitcast(F32R)[:, 2 * C:3 * C].rearrange("(ko p) m -> p ko m", p=P))
    projw_sb = const.tile([P, KO, C], F32R, tag="pw")
    nc.sync.dma_start(projw_sb[:], projw_d.bitcast(F32R).rearrange("(ko p) m -> p ko m", p=P))
    tw1_sb = const.tile([P, KO, HID], F32, tag="tw1")
    nc.sync.dma_start(tw1_sb[:], tw1_d.rearrange("(ko p) m -> p ko m", p=P))
    tw2_sb = const.tile([P, 3, 1], F32, tag="tw2")
    nc.sync.dma_start(tw2_sb[:], tw2_d.rearrange("(ko p) m -> p ko m", p=P))
    tb1_sb = const.tile([P, 3], F32, tag="tb1")
    nc.sync.dma_start(tb1_sb[:], tb1_d.rearrange("(ko p) -> p ko", p=P))
    b2_sb = const.tile([1, 1], F32, tag="b2")
    nc.sync.dma_start(b2_sb[:], tb2_d.unsqueeze(0))
    nb2_sb = const.tile([1, 1], F32, tag="nb2")
    nc.vector.tensor_scalar_mul(nb2_sb[:], b2_sb[:], -1.0)
    pb_bc = const.tile([P, C], F32, tag="pb")
    nc.sync.dma_start(pb_bc[:], projb_d.unsqueeze(0).to_broadcast([P, C]))
    ident = const.tile([P, P], F32, tag="id")
    make_identity(nc, ident[:])

    def phase_T(b):
        """x -> xT [128, KO, NPAD] via PE transpose."""
        xT = xT_pool.tile([P, KO, NPAD], F32R, tag="xT", name=f"xT_{b}")
        # zero the pad columns 577..580 (memset can't emit f32r ISA; use in*0 via DVE)
        nc.vector.tensor_scalar_mul(
            xT[:, :, N:NPAD],
            ident[:, 0:KO * (NPAD - N)].rearrange("p (a c) -> p a c", c=NPAD - N), 0.0)
        for mo in range(NT):
            rows = _rows(mo)
            xa = xa_p.tile([P, C], F32, tag="xa", name=f"xa_{b}_{mo}")
            nc.sync.dma_start(xa[:rows], x_d[b, mo * P:mo * P + rows, :])
            for ko in range(KO):
                pst = ps1.tile([P, 512], F32, tag="ps1", name=f"pst_{b}_{mo}_{ko}")
                nc.tensor.transpose(pst[:P, :rows], xa[:rows, ko * P:(ko + 1) * P],
                                    ident[:rows, :rows])
                nc.vector.tensor_copy(xT[:, ko, mo * P:mo * P + rows], pst[:P, :rows])
        return xT

    def phase_M(b, xT):
        """temperature MLP -> alpha_bc [128, 1]."""
        hsb = sm_p.tile([P, 3], F32, tag="hsb", name=f"hsb_{b}")
        for m3 in range(3):
            hps = ps1.tile([P, 512], F32, tag="ps1", name=f"hps_{b}_{m3}")
            for ko in range(KO):
                nc.tensor.matmul(hps[:, 0:1],
                                 lhsT=tw1_sb[:, ko, m3 * P:(m3 + 1) * P],
                                 rhs=xT.bitcast(F32)[:, ko, 0:1],
                                 start=(ko == 0), stop=(ko == KO - 1))
            nc.vector.tensor_scalar(hsb[:, m3:m3 + 1], hps[:, 0:1],
                                    tb1_sb[:, m3:m3 + 1], 0.0,
                                    op0=ALU.add, op1=ALU.max)
        sps = ps1.tile([P, 512], F32, tag="ps1", name=f"sps_{b}")
        for k3 in range(3):
            nc.tensor.matmul(sps[0:1, 0:1], lhsT=hsb[:, k3:k3 + 1],
                             rhs=tw2_sb[:, k3],
                             start=(k3 == 0), stop=(k3 == 2))
        esb = sm_p.tile([1, 1], F32, tag="esb", name=f"esb_{b}")
        # e = exp(-(s + b2)); sigmoid = 1/(1+e)
        nc.scalar.activation(esb[:], sps[0:1, 0:1], AF.Exp, bias=nb2_sb[:], scale=-1.0)
        dsb = sm_p.tile([1, 1], F32, tag="dsb", name=f"dsb_{b}")
        nc.vector.tensor_scalar_add(dsb[:], esb[:], 1.0)
        t2 = sm_p.tile([1, 1], F32, tag="t2", name=f"t2_{b}")
        nc.vector.reciprocal(t2[:], dsb[:])
        usb = sm_p.tile([1, 1], F32, tag="usb", name=f"usb_{b}")
        nc.vector.tensor_scalar(usb[:], t2[:], TMAX - TMIN, TMIN, op0=ALU.mult, op1=ALU.add)
        rsb = sm_p.tile([1, 1], F32, tag="rsb", name=f"rsb_{b}")
        nc.vector.reciprocal(rsb[:], usb[:])
        asb = sm_p.tile([1, 1], F32, tag="asb", name=f"asb_{b}")
        nc.vector.tensor_scalar_mul(asb[:], rsb[:], SCALE)  # alpha = scale / temp
        alpha_bc = sm_p.tile([P, 1], F32, tag="abc", name=f"abc_{b}")
        nc.gpsimd.partition_broadcast(alpha_bc[:], asb[:])
        if dbg is not None and b == 0:
            nc.sync.dma_start(dbg["alpha"], alpha_bc[:])
            nc.sync.dma_start(dbg["xT"], xT.bitcast(F32)[:])
        return alpha_bc

    def phase_QK(b, xT, alpha_bc):
        """qkT [128, 12, NPAD]; q rows (mo<6) scaled by alpha."""
        qkT = qkT_p.tile([P, 2 * KO, NPAD], F32R, tag="qkT", name=f"qkT_{b}")
        for mo in range(2 * KO):
            wt = wqk_p.tile([P, KO, P], F32R, tag="wqk", name=f"wt_{b}_{mo}")
            nc.sync.dma_start(wt[:], qkvw_d.bitcast(F32R)[:, mo * P:(mo + 1) * P]
                              .rearrange("(ko p) m -> p ko m", p=P))
            for i0 in ICH:
                pq = ps1.tile([P, 512], F32, tag="ps1", name=f"pq_{b}_{mo}_{i0}")
                for ko in range(KO):
                    nc.tensor.matmul(pq[:, :IW],
                                     lhsT=wt[:, ko],
                                     rhs=xT[:, ko, i0:i0 + IW],
                                     start=(ko == 0), stop=(ko == KO - 1))
                if mo < KO:
                    nc.vector.tensor_scalar_mul(qkT[:, mo, i0:i0 + IW], pq[:, :IW],
                                                alpha_bc[:])
                else:
                    nc.vector.tensor_copy(qkT[:, mo, i0:i0 + IW], pq[:, :IW])
        if dbg is not None and b == 0:
            nc.sync.dma_start(dbg["qkT"], qkT.bitcast(F32)[:])
        return qkT

    def gen_V(b, xT, out):
        """v_sb [128, NT, H*66]: ones col at 64, zeros at 65 of each head."""
        vsb = v_p.tile([P, NT, H * 66], F32R, tag="v", name=f"v_{b}")
        out["v"] = vsb
        for mo in range(NT):
            vv = vsb[:, mo].rearrange("p (h e) -> p h e", e=66)
            for (c0, cw) in CCH:
                pv = ps1.tile([P, 512], F32, tag="ps1", name=f"pv_{b}_{mo}_{c0}")
                for ko in range(KO):
                    nc.tensor.matmul(pv[:_mrows(mo), :cw],
                                     lhsT=xT[:, ko, mo * P:mo * P + _mrows(mo)],
                                     rhs=wv_sb[:, ko, c0:c0 + cw],
                                     start=(ko == 0), stop=(ko == KO - 1))
                nc.vector.tensor_copy(
                    vv[:_mrows(mo), c0 // 64:(c0 + cw) // 64, 0:64],
                    pv[:_mrows(mo), :cw].rearrange("p (h e) -> p h e", e=64))
            nc.vector.tensor_scalar(vv[:, :, 64], ident[:, 0:H], 0.0, 1.0,
                                    op0=ALU.mult, op1=ALU.add)
            nc.vector.tensor_scalar_mul(vv[:, :, 65], ident[:, 0:H], 0.0)
            yield
        if dbg is not None and b == 0:
            nc.sync.dma_start(dbg["v"], vsb.bitcast(F32)[:])

    def gen_A(b, qkT, vsb, out):
        """attention -> aT [128, KO, NPAD]; yields after each head pair."""
        aT = aT_pool.tile([P, KO, NPAD], F32R, tag="aT", name=f"aT_{b}")
        out["aT"] = aT
        for hp in range(H // 2):
            heads = (2 * hp, 2 * hp + 1)
            pTs = [pT_p.tile([P, NT, NPAD], F32R, tag="pT", name=f"pT_{b}_{hp}_{i}")
                   for i in range(2)]
            # S^T + exp; the head pair occupies PE row groups 0 / 64 and the
            # matmuls are interleaved so they run concurrently in the array.
            for jo in range(NT):
                jh = _mrows(jo)
                sp_pair = [ps2.tile([P, 2, 512], F32, tag="ps2", name=f"sp_{b}_{hp}_{jo}_{i}")
                           for i in range(2)]
                for ci, i0 in enumerate(ICH):
                    for hi, h in enumerate(heads):
                        moK, moQ, pp = KO + h // 2, h // 2, (h % 2) * 64
                        nc.tensor.matmul(
                            sp_pair[hi][:jh, ci, :IW],
                            lhsT=qkT[pp:pp + 64, moK, jo * P:jo * P + jh],
                            rhs=qkT[pp:pp + 64, moQ, i0:i0 + IW],
                            start=True, stop=True)
                for hi in range(2):
                    if jo == NT - 1:
                        # rows 64:66 pre-zeroed (base-64 aligned); exp then
                        # rewrites row 64 (j=576) and leaves pad row 65 zero
                        nc.vector.tensor_scalar_mul(pTs[hi][64:66, jo, :],
                                                    pb_bc[64:66, 0:NPAD], 0.0)
                    je = _rows(jo)
                    nc.scalar.activation(
                        pTs[hi][:je, jo].rearrange("p (c w) -> p c w", w=IW),
                        sp_pair[hi][:je, :, :IW], AF.Exp)
            # P @ V' (accumulate over j tiles; row 64 = sum of P = denominator)
            for hi, h in enumerate(heads):
                ochunks = []
                for ci, i0 in enumerate(ICH):
                    po = ps1.tile([P, 512], F32, tag="ps1", name=f"po_{b}_{h}_{ci}")
                    for jo in range(NT):
                        jh = _mrows(jo)
                        nc.tensor.matmul(
                            po[:66, :IW],
                            lhsT=vsb[:jh, jo, h * 66:(h + 1) * 66],
                            rhs=pTs[hi][:jh, jo, i0:i0 + IW],
                            start=(jo == 0), stop=(jo == NT - 1))
                    ochunks.append(po)
                # normalize: attnT rows (h%2)*64 .. +64 of c-tile h//2
                rden = rd_p.tile([P, NPAD], F32, tag="rd", name=f"rden_{b}_{h}")
                for ci, i0 in enumerate(ICH):
                    nc.vector.reciprocal(rden[64:65, i0:i0 + IW], ochunks[ci][64:65, :IW])
                # partition_broadcast requires its input at partition 0 (HW
                # reads garbage from base-64 APs) -> DMA-shift row 64 -> row 0
                rden0 = rd_p.tile([1, NPAD], F32, tag="rd0", name=f"rden0_{b}_{h}")
                nc.sync.dma_start(rden0[0:1, :], rden[64:65, :])
                rb = rb_p.tile([P, NPAD], F32, tag="rb", name=f"rb_{b}_{h}")
                nc.gpsimd.partition_broadcast(rb[:], rden0[0:1, :])
                mo6 = h // 2
                if h % 2 == 0:
                    for ci, i0 in enumerate(ICH):
                        nc.vector.tensor_mul(aT[0:64, mo6, i0:i0 + IW],
                                             ochunks[ci][0:64, :IW], rb[0:64, i0:i0 + IW])
                else:
                    tshift = tmp_p.tile([64, NPAD], F32R, tag="tmp", name=f"tsh_{b}_{h}")
                    for ci, i0 in enumerate(ICH):
                        nc.vector.tensor_mul(tshift[0:64, i0:i0 + IW],
                                             ochunks[ci][0:64, :IW], rb[0:64, i0:i0 + IW])
                    nc.sync.dma_start(aT[64:128, mo6, :], tshift[0:64, :])
            yield
        if dbg is not None and b == 0:
            nc.sync.dma_start(dbg["aT"], aT.bitcast(F32)[:])

    def gen_P(b, aT):
        """proj + bias -> y; yields after each n-tile."""
        for mo in range(NT):
            rows = _rows(mo)
            ysb = y_p.tile([P, C], F32, tag="y", name=f"y_{b}_{mo}")
            for (c0, cw) in CCH:
                pp_ = ps1.tile([P, 512], F32, tag="ps1", name=f"yp_{b}_{mo}_{c0}")
                for ko in range(KO):
                    nc.tensor.matmul(pp_[:_mrows(mo), :cw],
                                     lhsT=aT[:, ko, mo * P:mo * P + _mrows(mo)],
                                     rhs=projw_sb[:, ko, c0:c0 + cw],
                                     start=(ko == 0), stop=(ko == KO - 1))
                nc.vector.tensor_add(ysb[:rows, c0:c0 + cw], pp_[:rows, :cw],
                                     pb_bc[:rows, c0:c0 + cw])
            nc.sync.dma_start(y_d[b, mo * P:mo * P + rows, :], ysb[:rows])
            yield

    def drain(g):
        for _ in g:
            pass

    # Interleaved schedule: instance 1's V fills the PE while instance 0's
    # attention is ACT(exp)-bound, and instance 0's proj fills instance 1's
    # attention the same way.
    st0, st1 = {}, {}
    xT0 = phase_T(0)
    a0 = phase_M(0, xT0)
    qkT0 = phase_QK(0, xT0, a0)
    drain(gen_V(0, xT0, st0))
    xT1 = phase_T(1)
    a1 = phase_M(1, xT1)

    gA0 = gen_A(0, qkT0, st0["v"], st0)
    gV1 = gen_V(1, xT1, st1)
    while next(gA0, "end") != "end":
        next(gV1, None)
    drain(gV1)

    qkT1 = phase_QK(1, xT1, a1)

    gA1 = gen_A(1, qkT1, st1["v"], st1)
    gP0 = gen_P(0, st0["aT"])
    while next(gA1, "end") != "end":
        next(gP0, None)
    drain(gP0)
    drain(gen_P(1, st1["aT"]))


def build(debug=False):
    """Build and compile the per-core Bass module. Returns nc."""
    nc = bacc.Bacc("TRN2", target_bir_lowering=False, debug=False,
                   enable_asserts=False, num_devices=NCORES)
    io = {}
    io["x"] = nc.dram_tensor("x", [BPC, N, C], F32, kind="ExternalInput").ap()
    io["qkv_w"] = nc.dram_tensor("qkv_w", [C, 3 * C], F32, kind="ExternalInput").ap()
    io["proj_w"] = nc.dram_tensor("proj_w", [C, C], F32, kind="ExternalInput").ap()
    io["proj_b"] = nc.dram_tensor("proj_b", [C], F32, kind="ExternalInput").ap()
    io["t_w1"] = nc.dram_tensor("t_w1", [C, HID], F32, kind="ExternalInput").ap()
    io["t_b1"] = nc.dram_tensor("t_b1", [HID], F32, kind="ExternalInput").ap()
    io["t_w2"] = nc.dram_tensor("t_w2", [HID, 1], F32, kind="ExternalInput").ap()
    io["t_b2"] = nc.dram_tensor("t_b2", [1], F32, kind="ExternalInput").ap()
    io["y"] = nc.dram_tensor("y", [BPC, N, C], F32, kind="ExternalOutput").ap()

    dbg = None
    if debug:
        dbg = {
            "alpha": nc.dram_tensor("dbg_alpha", [P, 1], F32, kind="ExternalOutput").ap(),
            "xT": nc.dram_tensor("dbg_xT", [P, KO, NPAD], F32, kind="ExternalOutput").ap(),
            "qkT": nc.dram_tensor("dbg_qkT", [P, 2 * KO, NPAD], F32, kind="ExternalOutput").ap(),
            "v": nc.dram_tensor("dbg_v", [P, NT, H * 66], F32, kind="ExternalOutput").ap(),
            "aT": nc.dram_tensor("dbg_aT", [P, KO, NPAD], F32, kind="ExternalOutput").ap(),
        }
    with tile.TileContext(nc) as tc:
        _emit(tc, io, dbg)
    nc.compile()
    return nc


_NC_CACHE = None


def _get_nc():
    global _NC_CACHE
    if _NC_CACHE is None:
        _NC_CACHE = build()
    return _NC_CACHE


def make_in_maps(inputs: dict) -> list[dict]:
    ws = {k: np.ascontiguousarray(np.asarray(v, dtype=np.float32))
          for k, v in inputs.items() if k != "x"}
    x = np.asarray(inputs["x"], dtype=np.float32)
    return [dict(ws, x=np.ascontiguousarray(x[i * BPC:(i + 1) * BPC]))
            for i in range(NCORES)]


def kernel(**inputs) -> np.ndarray:
    from concourse.bass_utils import run_bass_kernel_spmd
    nc = _get_nc()
    in_maps = make_in_maps(inputs)
    res = run_bass_kernel_spmd(nc, in_maps, core_ids=list(range(NCORES)))
    return np.concatenate([r["y"] for r in res.results], axis=0)


if __name__ == "__main__":
    rng = np.random.default_rng(0)
    ins = {
        "x": rng.standard_normal((B, N, C), dtype=np.float32),
        "qkv_w": (rng.standard_normal((C, 3 * C)) * 0.02).astype(np.float32),
        "proj_w": (rng.standard_normal((C, C)) * 0.02).astype(np.float32),
        "proj_b": np.zeros(C, np.float32),
        "t_w1": (rng.standard_normal((C, HID)) * 0.02).astype(np.float32),
        "t_b1": np.zeros(HID, np.float32),
        "t_w2": (rng.standard_normal((HID, 1)) * 0.02).astype(np.float32),
        "t_b2": np.zeros(1, np.float32),
    }
    out = kernel(**ins)
    print("out", out.shape, out.dtype, float(np.abs(out).max()))
